# revision 31
# baseline (speedup 1.0000x reference)
"""Trainium2 Bass kernel for nn_MultiHeadAttention (B16 L1024 D512 H8).

Sharding: pure data-parallel, 2 batches per core across 8 NeuronCores.

Per-core device pipeline (per batch, per head):
  - QKV projections as f32r matmuls producing transposed layouts
    (qhT/khT = [dk, lq]; vh = [lk, dv] with an appended ones column).
  - Scores computed transposed: S^T[lk, lq] = khT.T @ qhT so the per-key
    time-decay bias is a per-partition ACT bias fused into the exp, and
    P^T feeds the P@V matmul directly with no transpose.
  - Mask applied on the f32 PSUM scores via copy_predicated(-1e30).
  - P@V uses vh with a ones column; the extra output row gives softmax
    row-sums for free.
  - Attention probabilities are transposed back per 128x128 tile on the
    PE, normalized during the PSUM->SBUF copy (per-partition reciprocal),
    and DMA'd out.
  - fc projection consumes the P@V output layout directly; fc bias added
    via a K=1 ones matmul; LayerNorm rstd = exp(-0.5*ln(var+eps)) so the
    whole kernel uses one ACT table set (natural_log_exp).
"""

import sys

sys.path.insert(0, "/opt/trn_rl_repo")

import numpy as np

import concourse.bass as bass
import concourse.bacc as bacc
import concourse.mybir as mybir
import concourse.tile as tile
from concourse import bass_utils

B, L, D, H, DK, DV = 16, 1024, 512, 8, 64, 64
LN_EPS = 1e-5
E = float(np.e)
NCORES = 8
BPC = B // NCORES  # batches per core

F32 = mybir.dt.float32
F32R = mybir.dt.float32r
U8 = mybir.dt.uint8
BF16 = mybir.dt.bfloat16
AF = mybir.ActivationFunctionType
OP = mybir.AluOpType
AX = mybir.AxisListType

_cache = {}


def _patch_act_tables():
    """Force every activation func onto the natural_log_exp set so the
    kernel needs exactly one ACT table load (Exp+Ln+Copy+Square all live
    there); the default chooser thrashes between exp-only and ln-only sets."""
    from concourse.hw_specs import get_activation_tables as _gat

    def single(arch):
        t = _gat(arch)
        return {
            k: (v if k == "natural_log_exp_and_others" else type(v)())
            for k, v in t.items()
        }

    bacc.get_activation_tables = single


def _build():
    _patch_act_tables()
    nc = bacc.Bacc("TRN2", target_bir_lowering=False, debug=False)

    dt_in = {}
    def din(name, shape, dt=F32):
        dt_in[name] = nc.dram_tensor(name, shape, dt, kind="ExternalInput").ap()
        return dt_in[name]

    qT_d = din("qT", (BPC, D, L))
    kT_d = din("kT", (BPC, D, L))
    vT_d = din("vT", (BPC, D, L), BF16)
    maskT_d = din("maskT", (BPC, L, L), BF16)
    tg_d = din("tgrid", (128, 128))
    tp_d = din("tp", (1, 1))
    tm_d = din("tm", (1, 1))
    wq_d = din("wqT", (D, H * DK))
    wk_d = din("wkT", (D, H * DK))
    wv_d = din("wvT", (D, H * DV), BF16)
    fcw_d = din("fcwT", (H * DV, D), BF16)
    fcb_d = din("fcb", (1, D), BF16)
    lng_d = din("lng", (1, D))
    lnb_d = din("lnb", (1, D))
    id_d = din("ident", (128, 128))
    idb_d = din("identb", (128, 128), BF16)
    onesr_d = din("onesrow", (1, 128))
    onesrb_d = din("onesrowb", (1, 128), BF16)

    out_d = nc.dram_tensor("out_o", (BPC, L, D), F32, kind="ExternalOutput").ap()
    attn_d = nc.dram_tensor("attn_o", (H, BPC, L, L), F32, kind="ExternalOutput").ap()

    with tile.TileContext(nc) as tc:
        with (
            tc.tile_pool(name="const", bufs=1) as cp,
            tc.tile_pool(name="small", bufs=6) as sp,
            tc.tile_pool(name="xt", bufs=2) as xp,
            tc.tile_pool(name="batch", bufs=1) as bp,
            tc.tile_pool(name="pt", bufs=4) as ptp,
            tc.tile_pool(name="stg", bufs=5) as stg,
            tc.tile_pool(name="rsp", bufs=2) as rsp,
            tc.tile_pool(name="scr", bufs=2) as scr,
            tc.tile_pool(name="psA", bufs=2, space="PSUM") as psA,
            tc.tile_pool(name="psB", bufs=2, space="PSUM") as psB,
            tc.tile_pool(name="psC", bufs=2, space="PSUM") as psC,
        ):
            # ---- constants ----
            identf = cp.tile([128, 128], F32)
            nc.scalar.dma_start(identf, id_d)
            onesrow = cp.tile([1, 128], F32)
            nc.scalar.dma_start(onesrow, onesr_d)
            onescol = cp.tile([128, 1], F32)
            nc.scalar.dma_start(onescol, onesr_d.rearrange("a b -> b a"))
            tg = cp.tile([128, 128], F32)
            nc.scalar.dma_start(tg, tg_d)
            tp_sb = sp.tile([1, 1], F32)
            nc.scalar.dma_start(tp_sb, tp_d)
            tm_sb = sp.tile([1, 1], F32)
            nc.scalar.dma_start(tm_sb, tm_d)
            wq = cp.tile([128, 4, 512], F32R)
            wk = cp.tile([128, 4, 512], F32R)
            for t, d in ((wq, wq_d), (wk, wk_d)):
                nc.scalar.dma_start(t, d.rearrange("(c p) n -> p c n", p=128).bitcast(F32R))
            wv = cp.tile([128, 4, 512], BF16)
            nc.scalar.dma_start(wv, wv_d.rearrange("(c p) n -> p c n", p=128))
            fcw = cp.tile([128, 4, 512], BF16)
            nc.scalar.dma_start(fcw, fcw_d.rearrange("(c p) n -> p c n", p=128))
            identb = cp.tile([128, 128], BF16)
            nc.scalar.dma_start(identb, idb_d)
            onesrowb = cp.tile([1, 128], BF16)
            nc.scalar.dma_start(onesrowb, onesrb_d)
            fcb = cp.tile([1, 512], BF16)
            nc.scalar.dma_start(fcb, fcb_d)
            lng_b = cp.tile([128, 512], F32)
            lnb_b = cp.tile([128, 512], F32)
            for t, d in ((lng_b, lng_d), (lnb_b, lnb_d)):
                nc.scalar.dma_start(
                    t, bass.AP(tensor=d.tensor, offset=d.offset, ap=[[0, 128]] + d.ap[1:])
                )
            eps = cp.tile([128, 1], F32)
            nc.vector.memset(eps, LN_EPS)

            # ---- phase A: time-decay bias grid ----
            rs = sp.tile([128, 1], F32)
            nc.vector.tensor_reduce(rs, tg, axis=AX.X, op=OP.add)
            rmx = sp.tile([128, 1], F32)
            nc.vector.tensor_reduce(rmx, tg, axis=AX.X, op=OP.max)
            pstot = psC.tile([1, 1], F32, tag="tr")
            nc.tensor.matmul(pstot, rs, onescol, start=True, stop=True)
            tot = sp.tile([1, 1], F32)
            nc.vector.tensor_copy(tot, pstot)
            psmx = psC.tile([1, 128], F32, tag="tr")
            nc.tensor.transpose(psmx, rmx, identf)
            mx1 = sp.tile([1, 1], F32)
            nc.vector.tensor_reduce(mx1, psmx, axis=AX.X, op=OP.max)
            rtot = sp.tile([1, 1], F32)
            nc.vector.reciprocal(rtot, tot)
            invmean = sp.tile([1, 1], F32)
            nc.vector.tensor_scalar(invmean, rtot, float(B * L), None, OP.mult)
            tdmax = sp.tile([1, 1], F32)
            nc.vector.tensor_mul(tdmax, mx1, invmean)
            # softplus(tp) = ln(1 + exp(tp))
            e1 = sp.tile([1, 1], F32)
            nc.scalar.activation(e1, tp_sb, AF.Exp)
            e2 = sp.tile([1, 1], F32)
            nc.vector.tensor_scalar(e2, e1, 1.0, None, OP.add)
            spl = sp.tile([1, 1], F32)
            nc.scalar.activation(spl, e2, AF.Ln)
            ace = sp.tile([1, 1], F32)
            nc.vector.tensor_mul(ace, spl, tdmax)
            nc.vector.tensor_scalar(ace, ace, E, None, OP.add)

            def bcast128(src):
                ps = psC.tile([128, 1], F32, tag="tr")
                nc.tensor.matmul(ps, onesrow, src, start=True, stop=True)
                dst = sp.tile([128, 1], F32)
                nc.scalar.copy(dst, ps)
                return dst

            invmean_b = bcast128(invmean)
            ace_b = bcast128(ace)
            tm_b = bcast128(tm_sb)

            lnu = cp.tile([128, 128], F32)
            nc.scalar.activation(lnu, tg, AF.Ln, bias=ace_b, scale=invmean_b)
            binv = cp.tile([128, 128], F32)
            nc.vector.reciprocal(binv, lnu)
            bgrid = cp.tile([128, 128], F32)
            nc.vector.tensor_scalar(bgrid, binv, tm_b, None, OP.mult)
            psbt = psC.tile([128, 128], F32, tag="tr")
            nc.tensor.transpose(psbt, bgrid, identf)
            biasT = cp.tile([128, 128], F32)
            nc.scalar.copy(biasT, psbt)

            # ---- per-batch, software-pipelined phases ----
            bt = {}

            def emit_proj(bl):
                qhT = bp.tile([128, 4, 1024], F32R, tag="qhT", name=f"qhT{bl}")
                khT = bp.tile([128, 4, 1024], F32R, tag="khT", name=f"khT{bl}")
                vh = bp.tile([128, 8, 8, 65], BF16, tag="vh", name=f"vh{bl}")
                maskT = bp.tile([128, 8, 1024], BF16, tag="maskT", name=f"maskT{bl}")
                outT = bp.tile([128, 4, 1024], BF16, tag="outT", name=f"outT{bl}")
                xn = bp.tile([128, 8, 512], BF16, tag="xn", bufs=2, name=f"xn{bl}")
                bt[bl] = dict(qhT=qhT, khT=khT, vh=vh, maskT=maskT, outT=outT, xn=xn)
                mask_r = maskT_d[bl].rearrange("(m p) l -> p m l", p=128)
                for m in range(8):
                    nc.sync.dma_start(maskT[:, m, :], mask_r[:, m, :])

                for src_d, w_sb, dst in ((qT_d, wq, qhT),):
                    xin = xp.tile([128, 4, 1024], F32R, tag="xt", name=f"x{bl}_{dst.tensor.name}")
                    src_r = src_d[bl].rearrange("(c p) l -> p c l", p=128).bitcast(F32R)
                    for kc in range(4):
                        nc.sync.dma_start(xin[:, kc, :], src_r[:, kc, :])
                    for m in range(4):
                        ps = psA.tile([128, 1024], F32, tag="s")
                        for n in range(2):
                            for kc in range(4):
                                nc.tensor.matmul(
                                    ps[:, n * 512 : (n + 1) * 512],
                                    w_sb[:, kc, m * 128 : (m + 1) * 128],
                                    xin[:, kc, n * 512 : (n + 1) * 512],
                                    start=(kc == 0),
                                    stop=(kc == 3),
                                )
                        nc.vector.tensor_scalar(
                            dst[:, m, :], ps, 1.0, None, OP.mult
                        )

                xin = xp.tile([128, 4, 1024], F32R, tag="xt", name=f"x{bl}_k")
                src_r = kT_d[bl].rearrange("(c p) l -> p c l", p=128).bitcast(F32R)
                for kc in range(4):
                    nc.sync.dma_start(xin[:, kc, :], src_r[:, kc, :])
                for m in range(4):
                    for n in range(2):
                        ps = psB.tile([128, 512], F32, tag="av")
                        for kc in range(4):
                            nc.tensor.matmul(
                                ps,
                                wk[:, kc, m * 128 : (m + 1) * 128],
                                xin[:, kc, n * 512 : (n + 1) * 512],
                                start=(kc == 0),
                                stop=(kc == 3),
                            )
                        nc.vector.tensor_scalar(
                            khT[:, m, n * 512 : (n + 1) * 512], ps, 1.0,
                            None, OP.mult,
                        )

                xin = xp.tile([128, 4, 1024], BF16, tag="xt", name=f"x{bl}_v")
                src_r = vT_d[bl].rearrange("(c p) l -> p c l", p=128)
                for kc in range(4):
                    nc.sync.dma_start(xin[:, kc, :], src_r[:, kc, :])
                for m in range(8):
                    ps = psB.tile([128, 512], F32, tag="av")
                    for kc in range(4):
                        nc.tensor.matmul(
                            ps,
                            xin[:, kc, m * 128 : (m + 1) * 128],
                            wv[:, kc, :],
                            start=(kc == 0),
                            stop=(kc == 3),
                        )
                    # scatter heads into [h, 65] layout (col 64 = ones)
                    nc.vector.tensor_scalar(
                        vh[:, m, :, 0:64],
                        ps.rearrange("p (h e) -> p h e", h=8),
                        1.0,
                        None,
                        OP.mult,
                    )
                    nc.vector.memset(vh[:, m, :, 64:65], 1.0)

            def emit_heads(bl, post=None):
                b = bt[bl]
                for h in range(8):
                    if post is not None:
                        post(h)
                    po = (h % 2) * 64
                    ch = h // 2
                    qh = b['qhT'][po : po + 64, ch, :]
                    kh = b['khT'][po : po + 64, ch, :]

                    recip = rsp.tile([128, 8], F32, tag="recip", name=f"rc{bl}{h}")
                    pav = [
                        psB.tile([65, 512], F32, tag="av", name=f"pav{bl}{h}_{_n}")
                        for _n in range(2)
                    ]

                    ptgs = []
                    for mg in range(2):
                        ptg = ptp.tile([128, 4, 1024], BF16, tag="pt",
                                       name=f"ptg{bl}{h}_{mg}")
                        ptgs.append(ptg)
                        for ml in range(4):
                            m = mg * 4 + ml
                            ps = psA.tile([128, 1024], F32, tag="s")
                            for n in range(2):
                                nc.tensor.matmul(
                                    ps[:, n * 512 : (n + 1) * 512],
                                    kh[:, m * 128 : (m + 1) * 128],
                                    qh[:, n * 512 : (n + 1) * 512],
                                    start=True,
                                    stop=True,
                                )
                            nc.scalar.activation(
                                ptg[:, ml, :],
                                ps,
                                AF.Exp,
                                bias=biasT[:, bl * 8 + m : bl * 8 + m + 1],
                                scale=0.125,
                            )
                            nc.vector.tensor_mul(
                                ptg[:, ml, :], ptg[:, ml, :], b['maskT'][:, m, :]
                            )
                        for ml in range(4):
                            m = mg * 4 + ml
                            for n in range(2):
                                nc.tensor.matmul(
                                    pav[n],
                                    b['vh'][:, m, h, :],
                                    ptg[:, ml, n * 512 : (n + 1) * 512],
                                    start=(m == 0),
                                    stop=(m == 7),
                                )

                    avs = rsp.tile([65, 1024], F32, tag="avs", name=f"avs{bl}{h}")
                    for n in range(2):
                        nc.vector.tensor_copy(
                            avs[:, n * 512 : (n + 1) * 512], pav[n]
                        )

                    # transpose av result per lq-tile (4 tiles per PSUM group):
                    # [65, 128] -> [128, 65]; col 64 holds softmax row-sums
                    for tg4 in range(2):
                        ptx = psC.tile([128, 4, 65], F32, tag="tr",
                                       name=f"ptx{bl}{h}_{tg4}")
                        for tt in range(4):
                            t = tg4 * 4 + tt
                            nc.tensor.transpose(
                                ptx[:, tt, :],
                                avs[:, t * 128 : (t + 1) * 128],
                                identf[0:65, 0:65],
                            )
                        nc.vector.reciprocal(
                            recip[:, tg4 * 4 : (tg4 + 1) * 4],
                            ptx[:, :, 64],
                        )
                        for tt in range(4):
                            t = tg4 * 4 + tt
                            nc.vector.tensor_scalar(
                                b['xn'][:, t, h * 64 : (h + 1) * 64],
                                ptx[:, tt, 0:64],
                                recip[:, t : t + 1],
                                None,
                                OP.mult,
                            )

                    # transpose P^T back (8 bf16 tiles pack into one PSUM
                    # bank), normalize with the row-sum reciprocal during the
                    # PSUM->SBUF copy, and flush to HBM
                    for t in range(8):
                        pstB = psC.tile([128, 1024], BF16, tag="tr",
                                        padded_shape=[128, 1024],
                                        name=f"pst{bl}{h}_{t}")
                        for i in range(8):
                            nc.tensor.transpose(
                                pstB[:, i * 128 : (i + 1) * 128],
                                ptgs[i // 4][:, i % 4, t * 128 : (t + 1) * 128],
                                identb,
                            )
                        st = stg.tile([128, 1024], BF16, tag="stage",
                                      name=f"st{bl}{h}_{t}")
                        if t % 2 == 0:
                            nc.vector.tensor_scalar(
                                st, pstB, recip[:, t : t + 1], None, OP.mult
                            )
                        else:
                            nc.scalar.mul(st, pstB, recip[:, t : t + 1])
                        nc.gpsimd.dma_start(
                            attn_d[h, bl, t * 128 : (t + 1) * 128, :], st
                        )

            def emit_xretr(bl, j, grp):
                b = bt[bl]
                pst = psA.tile([128, 512], BF16, tag="s",
                               padded_shape=[128, 512], name=f"xr{bl}_{j}_{grp}")
                for tt in range(4):
                    t = grp * 4 + tt
                    nc.tensor.transpose(
                        pst[:, tt * 128 : (tt + 1) * 128],
                        b['xn'][:, t, j * 128 : (j + 1) * 128],
                        identb,
                    )
                nc.scalar.copy(b['outT'][:, j, grp * 512 : (grp + 1) * 512], pst)

            def emit_fc_tile(bl, t):
                b = bt[bl]
                psf = psC.tile([128, 512], F32, tag="tr", name=f"psf{bl}_{t}")
                for j in range(4):
                    nc.tensor.matmul(
                        psf,
                        b['outT'][:, j, t * 128 : (t + 1) * 128],
                        fcw[:, j, :],
                        start=(j == 0),
                        stop=False,
                    )
                nc.tensor.matmul(psf, onesrowb, fcb, start=False, stop=True)

                fco = scr.tile([128, 512], F32, tag="fco", name=f"fco{bl}_{t}")
                nc.scalar.copy(fco, psf)
                sum_t = sp.tile([128, 1], F32, tag="ln", name=f"s{bl}_{t}")
                nc.vector.tensor_reduce(sum_t, psf, axis=AX.X, op=OP.add)
                sumsq = sp.tile([128, 1], F32, tag="ln", name=f"q{bl}_{t}")
                nc.scalar.activation(psf, psf, AF.Square, accum_out=sumsq)
                mean_t = sp.tile([128, 1], F32, tag="ln", name=f"m{bl}_{t}")
                nc.vector.tensor_scalar(mean_t, sum_t, 1.0 / 512.0, None, OP.mult)
                m2 = sp.tile([128, 1], F32, tag="ln", name=f"m2{bl}_{t}")
                nc.vector.tensor_mul(m2, mean_t, mean_t)
                var_t = sp.tile([128, 1], F32, tag="ln", name=f"v{bl}_{t}")
                nc.vector.scalar_tensor_tensor(
                    var_t, sumsq, 1.0 / 512.0, m2, OP.mult, OP.subtract
                )
                t1 = sp.tile([128, 1], F32, tag="ln", name=f"t{bl}_{t}")
                nc.scalar.activation(t1, var_t, AF.Ln, bias=eps, scale=1.0)
                rstd = sp.tile([128, 1], F32, tag="ln", name=f"r{bl}_{t}")
                nc.scalar.activation(rstd, t1, AF.Exp, scale=-0.5)
                y1 = scr.tile([128, 512], F32, tag="y1", name=f"y1{bl}_{t}")
                nc.vector.scalar_tensor_tensor(
                    y1, fco, mean_t, lng_b, OP.subtract, OP.mult
                )
                y2 = scr.tile([128, 512], F32, tag="y1", name=f"y2{bl}_{t}")
                nc.vector.scalar_tensor_tensor(
                    y2, y1, rstd, lnb_b, OP.mult, OP.add
                )
                nc.sync.dma_start(out_d[bl, t * 128 : (t + 1) * 128, :], y2)

            def emit_fc(bl):
                for j in range(4):
                    for grp in range(2):
                        emit_xretr(bl, j, grp)
                for t in range(8):
                    emit_fc_tile(bl, t)

            emit_proj(0)
            emit_heads(0)
            emit_proj(1)

            emit_fc(0)
            emit_heads(1)
            emit_fc(1)

    nc.compile()
    return nc


def kernel(**inputs):
    inp = {k: np.asarray(v) for k, v in inputs.items()}
    q, k, v = inp["q"], inp["k"], inp["v"]
    mask = inp["mask"]
    td = inp["time_diff"].astype(np.float32)

    qT = np.ascontiguousarray(q.astype(np.float32).transpose(0, 2, 1))
    kT = np.ascontiguousarray(k.astype(np.float32).transpose(0, 2, 1))
    import ml_dtypes as _mld
    vT = np.ascontiguousarray(v.transpose(0, 2, 1).astype(_mld.bfloat16))
    import ml_dtypes
    maskT = np.ascontiguousarray(
        (~mask).transpose(0, 2, 1).astype(ml_dtypes.bfloat16)
    )
    tgrid = np.ascontiguousarray(td.reshape(128, 128))

    common = {
        "tp": inp["time_plus"].astype(np.float32).reshape(1, 1),
        "tm": inp["time_mul"].astype(np.float32).reshape(1, 1),
        "wqT": np.ascontiguousarray(inp["Wq"].astype(np.float32).T),
        "wkT": np.ascontiguousarray(inp["Wk"].astype(np.float32).T),
        "wvT": np.ascontiguousarray(inp["Wv"].T.astype(_mld.bfloat16)),
        "fcwT": np.ascontiguousarray(inp["fc_w"].T.astype(ml_dtypes.bfloat16)),
        "fcb": inp["fc_b"].astype(ml_dtypes.bfloat16).reshape(1, D),
        "lng": inp["ln_g"].astype(np.float32).reshape(1, D),
        "lnb": inp["ln_b"].astype(np.float32).reshape(1, D),
        "ident": np.eye(128, dtype=np.float32),
        "identb": np.eye(128, dtype=ml_dtypes.bfloat16),
        "onesrow": np.ones((1, 128), dtype=np.float32),
        "onesrowb": np.ones((1, 128), dtype=ml_dtypes.bfloat16),
    }

    if "nc" not in _cache:
        _cache["nc"] = _build()
    nc = _cache["nc"]

    in_maps = []
    for c in range(NCORES):
        s = slice(c * BPC, (c + 1) * BPC)
        order = list(range(c * BPC, (c + 1) * BPC)) + [
            b for b in range(B) if not (c * BPC <= b < (c + 1) * BPC)
        ]
        in_maps.append(
            {
                "qT": qT[s],
                "kT": kT[s],
                "vT": vT[s],
                "maskT": maskT[s],
                "tgrid": np.ascontiguousarray(td[order].reshape(128, 128)),
                **common,
            }
        )

    res = bass_utils.run_bass_kernel_spmd(nc, in_maps, core_ids=list(range(NCORES)))

    out = np.empty((B, L, D), np.float32)
    attn = np.empty((H, B, L, L), np.float32)
    for c in range(NCORES):
        r = res.results[c]
        out[c * BPC : (c + 1) * BPC] = r["out_o"]
        attn[:, c * BPC : (c + 1) * BPC] = r["attn_o"]
    return out, attn.reshape(H * B, L, L)


# revision 32
# speedup vs baseline: 1.0333x; 1.0333x over previous
"""Trainium2 Bass kernel for nn_MultiHeadAttention (B16 L1024 D512 H8).

Sharding: pure data-parallel, 2 batches per core across 8 NeuronCores.

Per-core device pipeline (per batch, per head):
  - QKV projections as f32r matmuls producing transposed layouts
    (qhT/khT = [dk, lq]; vh = [lk, dv] with an appended ones column).
  - Scores computed transposed: S^T[lk, lq] = khT.T @ qhT so the per-key
    time-decay bias is a per-partition ACT bias fused into the exp, and
    P^T feeds the P@V matmul directly with no transpose.
  - Mask applied on the f32 PSUM scores via copy_predicated(-1e30).
  - P@V uses vh with a ones column; the extra output row gives softmax
    row-sums for free.
  - Attention probabilities are transposed back per 128x128 tile on the
    PE, normalized during the PSUM->SBUF copy (per-partition reciprocal),
    and DMA'd out.
  - fc projection consumes the P@V output layout directly; fc bias added
    via a K=1 ones matmul; LayerNorm rstd = exp(-0.5*ln(var+eps)) so the
    whole kernel uses one ACT table set (natural_log_exp).
"""

import sys

sys.path.insert(0, "/opt/trn_rl_repo")

import numpy as np

import concourse.bass as bass
import concourse.bacc as bacc
import concourse.mybir as mybir
import concourse.tile as tile
from concourse import bass_utils

B, L, D, H, DK, DV = 16, 1024, 512, 8, 64, 64
LN_EPS = 1e-5
E = float(np.e)
NCORES = 8
BPC = B // NCORES  # batches per core

F32 = mybir.dt.float32
F32R = mybir.dt.float32r
U8 = mybir.dt.uint8
BF16 = mybir.dt.bfloat16
AF = mybir.ActivationFunctionType
OP = mybir.AluOpType
AX = mybir.AxisListType

_cache = {}


def _patch_act_tables():
    """Force every activation func onto the natural_log_exp set so the
    kernel needs exactly one ACT table load (Exp+Ln+Copy+Square all live
    there); the default chooser thrashes between exp-only and ln-only sets."""
    from concourse.hw_specs import get_activation_tables as _gat

    def single(arch):
        t = _gat(arch)
        return {
            k: (v if k == "natural_log_exp_and_others" else type(v)())
            for k, v in t.items()
        }

    bacc.get_activation_tables = single


def _build():
    _patch_act_tables()
    nc = bacc.Bacc("TRN2", target_bir_lowering=False, debug=False)

    dt_in = {}
    def din(name, shape, dt=F32):
        dt_in[name] = nc.dram_tensor(name, shape, dt, kind="ExternalInput").ap()
        return dt_in[name]

    qT_d = din("qT", (BPC, D, L))
    kT_d = din("kT", (BPC, D, L))
    vT_d = din("vT", (BPC, D, L), BF16)
    maskT_d = din("maskT", (BPC, L, L), BF16)
    tg_d = din("tgrid", (128, 128))
    tp_d = din("tp", (1, 1))
    tm_d = din("tm", (1, 1))
    wq_d = din("wqT", (D, H * DK))
    wk_d = din("wkT", (D, H * DK))
    wv_d = din("wvT", (D, H * DV), BF16)
    fcw_d = din("fcwT", (H * DV, D), BF16)
    fcb_d = din("fcb", (1, D), BF16)
    lng_d = din("lng", (1, D), BF16)
    lnb_d = din("lnb", (1, D), BF16)
    id_d = din("ident", (128, 128))
    idb_d = din("identb", (128, 128), BF16)
    onesr_d = din("onesrow", (1, 128))
    onesrb_d = din("onesrowb", (1, 128), BF16)

    out_d = nc.dram_tensor("out_o", (BPC, L, D), F32, kind="ExternalOutput").ap()
    attn_d = nc.dram_tensor("attn_o", (H, BPC, L, L), F32, kind="ExternalOutput").ap()

    with tile.TileContext(nc) as tc:
        with (
            tc.tile_pool(name="const", bufs=1) as cp,
            tc.tile_pool(name="small", bufs=8) as sp,
            tc.tile_pool(name="xt", bufs=2) as xp,
            tc.tile_pool(name="batch", bufs=1) as bp,
            tc.tile_pool(name="pt", bufs=4) as ptp,
            tc.tile_pool(name="stg", bufs=6) as stg,
            tc.tile_pool(name="rsp", bufs=2) as rsp,
            tc.tile_pool(name="scr", bufs=2) as scr,
            tc.tile_pool(name="psA", bufs=2, space="PSUM") as psA,
            tc.tile_pool(name="psB", bufs=2, space="PSUM") as psB,
            tc.tile_pool(name="psC", bufs=2, space="PSUM") as psC,
        ):
            # ---- constants ----
            identf = cp.tile([128, 128], F32)
            nc.scalar.dma_start(identf, id_d)
            onesrow = cp.tile([1, 128], F32)
            nc.scalar.dma_start(onesrow, onesr_d)
            onescol = cp.tile([128, 1], F32)
            nc.scalar.dma_start(onescol, onesr_d.rearrange("a b -> b a"))
            tg = cp.tile([128, 128], F32)
            nc.scalar.dma_start(tg, tg_d)
            tp_sb = sp.tile([1, 1], F32)
            nc.scalar.dma_start(tp_sb, tp_d)
            tm_sb = sp.tile([1, 1], F32)
            nc.scalar.dma_start(tm_sb, tm_d)
            wq = cp.tile([128, 4, 512], F32R)
            wk = cp.tile([128, 4, 512], F32R)
            for t, d in ((wq, wq_d), (wk, wk_d)):
                nc.scalar.dma_start(t, d.rearrange("(c p) n -> p c n", p=128).bitcast(F32R))
            wv = cp.tile([128, 4, 512], BF16)
            nc.scalar.dma_start(wv, wv_d.rearrange("(c p) n -> p c n", p=128))
            fcw = cp.tile([128, 4, 512], BF16)
            nc.scalar.dma_start(fcw, fcw_d.rearrange("(c p) n -> p c n", p=128))
            identb = cp.tile([128, 128], BF16)
            nc.scalar.dma_start(identb, idb_d)
            onesrowb = cp.tile([1, 128], BF16)
            nc.scalar.dma_start(onesrowb, onesrb_d)
            fcb = cp.tile([1, 512], BF16)
            nc.scalar.dma_start(fcb, fcb_d)
            lng_b = cp.tile([128, 512], BF16)
            lnb_b = cp.tile([128, 512], BF16)
            for t, d in ((lng_b, lng_d), (lnb_b, lnb_d)):
                nc.scalar.dma_start(
                    t, bass.AP(tensor=d.tensor, offset=d.offset, ap=[[0, 128]] + d.ap[1:])
                )
            eps = cp.tile([128, 1], F32)
            nc.vector.memset(eps, LN_EPS)

            # ---- phase A: time-decay bias grid ----
            rs = sp.tile([128, 1], F32)
            nc.vector.tensor_reduce(rs, tg, axis=AX.X, op=OP.add)
            rmx = sp.tile([128, 1], F32)
            nc.vector.tensor_reduce(rmx, tg, axis=AX.X, op=OP.max)
            pstot = psC.tile([1, 1], F32, tag="tr")
            nc.tensor.matmul(pstot, rs, onescol, start=True, stop=True)
            tot = sp.tile([1, 1], F32)
            nc.vector.tensor_copy(tot, pstot)
            psmx = psC.tile([1, 128], F32, tag="tr")
            nc.tensor.transpose(psmx, rmx, identf)
            mx1 = sp.tile([1, 1], F32)
            nc.vector.tensor_reduce(mx1, psmx, axis=AX.X, op=OP.max)
            rtot = sp.tile([1, 1], F32)
            nc.vector.reciprocal(rtot, tot)
            invmean = sp.tile([1, 1], F32)
            nc.vector.tensor_scalar(invmean, rtot, float(B * L), None, OP.mult)
            tdmax = sp.tile([1, 1], F32)
            nc.vector.tensor_mul(tdmax, mx1, invmean)
            # softplus(tp) = ln(1 + exp(tp))
            e1 = sp.tile([1, 1], F32)
            nc.scalar.activation(e1, tp_sb, AF.Exp)
            e2 = sp.tile([1, 1], F32)
            nc.vector.tensor_scalar(e2, e1, 1.0, None, OP.add)
            spl = sp.tile([1, 1], F32)
            nc.scalar.activation(spl, e2, AF.Ln)
            ace = sp.tile([1, 1], F32)
            nc.vector.tensor_mul(ace, spl, tdmax)
            nc.vector.tensor_scalar(ace, ace, E, None, OP.add)

            def bcast128(src):
                ps = psC.tile([128, 1], F32, tag="tr")
                nc.tensor.matmul(ps, onesrow, src, start=True, stop=True)
                dst = sp.tile([128, 1], F32)
                nc.scalar.copy(dst, ps)
                return dst

            invmean_b = bcast128(invmean)
            ace_b = bcast128(ace)
            tm_b = bcast128(tm_sb)

            lnu = cp.tile([128, 128], F32)
            nc.scalar.activation(lnu, tg, AF.Ln, bias=ace_b, scale=invmean_b)
            binv = cp.tile([128, 128], F32)
            nc.vector.reciprocal(binv, lnu)
            bgrid = cp.tile([128, 128], F32)
            nc.vector.tensor_scalar(bgrid, binv, tm_b, None, OP.mult)
            psbt = psC.tile([128, 128], F32, tag="tr")
            nc.tensor.transpose(psbt, bgrid, identf)
            biasT = cp.tile([128, 128], F32)
            nc.scalar.copy(biasT, psbt)

            # ---- per-batch, software-pipelined phases ----
            bt = {}

            def emit_proj(bl):
                qhT = bp.tile([128, 4, 1024], F32R, tag="qhT", name=f"qhT{bl}")
                khT = bp.tile([128, 4, 1024], F32R, tag="khT", name=f"khT{bl}")
                vh = bp.tile([128, 8, 8, 65], BF16, tag="vh", name=f"vh{bl}")
                maskT = bp.tile([128, 8, 1024], BF16, tag="maskT", name=f"maskT{bl}")
                outT = bp.tile([128, 4, 1024], BF16, tag="outT", name=f"outT{bl}")
                xn = bp.tile([128, 8, 512], BF16, tag="xn", bufs=2, name=f"xn{bl}")
                bt[bl] = dict(qhT=qhT, khT=khT, vh=vh, maskT=maskT, outT=outT, xn=xn)
                mask_r = maskT_d[bl].rearrange("(m p) l -> p m l", p=128)
                for m in range(8):
                    nc.sync.dma_start(maskT[:, m, :], mask_r[:, m, :])

                for src_d, w_sb, dst in ((qT_d, wq, qhT),):
                    xin = xp.tile([128, 4, 1024], F32R, tag="xt", name=f"x{bl}_{dst.tensor.name}")
                    src_r = src_d[bl].rearrange("(c p) l -> p c l", p=128).bitcast(F32R)
                    for kc in range(4):
                        nc.sync.dma_start(xin[:, kc, :], src_r[:, kc, :])
                    for m in range(4):
                        ps = psA.tile([128, 1024], F32, tag="s")
                        for n in range(2):
                            for kc in range(4):
                                nc.tensor.matmul(
                                    ps[:, n * 512 : (n + 1) * 512],
                                    w_sb[:, kc, m * 128 : (m + 1) * 128],
                                    xin[:, kc, n * 512 : (n + 1) * 512],
                                    start=(kc == 0),
                                    stop=(kc == 3),
                                )
                        nc.vector.tensor_scalar(
                            dst[:, m, :], ps, 1.0, None, OP.mult
                        )

                xin = xp.tile([128, 4, 1024], F32R, tag="xt", name=f"x{bl}_k")
                src_r = kT_d[bl].rearrange("(c p) l -> p c l", p=128).bitcast(F32R)
                for kc in range(4):
                    nc.sync.dma_start(xin[:, kc, :], src_r[:, kc, :])
                for m in range(4):
                    for n in range(2):
                        ps = psB.tile([128, 512], F32, tag="av")
                        for kc in range(4):
                            nc.tensor.matmul(
                                ps,
                                wk[:, kc, m * 128 : (m + 1) * 128],
                                xin[:, kc, n * 512 : (n + 1) * 512],
                                start=(kc == 0),
                                stop=(kc == 3),
                            )
                        nc.vector.tensor_scalar(
                            khT[:, m, n * 512 : (n + 1) * 512], ps, 1.0,
                            None, OP.mult,
                        )

                xin = xp.tile([128, 4, 1024], BF16, tag="xt", name=f"x{bl}_v")
                src_r = vT_d[bl].rearrange("(c p) l -> p c l", p=128)
                for kc in range(4):
                    nc.sync.dma_start(xin[:, kc, :], src_r[:, kc, :])
                for m in range(8):
                    ps = psB.tile([128, 512], F32, tag="av")
                    for kc in range(4):
                        nc.tensor.matmul(
                            ps,
                            xin[:, kc, m * 128 : (m + 1) * 128],
                            wv[:, kc, :],
                            start=(kc == 0),
                            stop=(kc == 3),
                        )
                    # scatter heads into [h, 65] layout (col 64 = ones)
                    nc.vector.tensor_scalar(
                        vh[:, m, :, 0:64],
                        ps.rearrange("p (h e) -> p h e", h=8),
                        1.0,
                        None,
                        OP.mult,
                    )
                    nc.vector.memset(vh[:, m, :, 64:65], 1.0)

            def emit_heads(bl, post=None):
                b = bt[bl]
                for h in range(8):
                    if post is not None:
                        post(h)
                    po = (h % 2) * 64
                    ch = h // 2
                    qh = b['qhT'][po : po + 64, ch, :]
                    kh = b['khT'][po : po + 64, ch, :]

                    recip = rsp.tile([128, 8], F32, tag="recip", name=f"rc{bl}{h}")
                    pav = [
                        psB.tile([65, 512], F32, tag="av", name=f"pav{bl}{h}_{_n}")
                        for _n in range(2)
                    ]

                    ptgs = []
                    for mg in range(2):
                        ptg = ptp.tile([128, 4, 1024], BF16, tag="pt",
                                       name=f"ptg{bl}{h}_{mg}")
                        ptgs.append(ptg)
                        for ml in range(4):
                            m = mg * 4 + ml
                            ps = psA.tile([128, 1024], F32, tag="s")
                            for n in range(2):
                                nc.tensor.matmul(
                                    ps[:, n * 512 : (n + 1) * 512],
                                    kh[:, m * 128 : (m + 1) * 128],
                                    qh[:, n * 512 : (n + 1) * 512],
                                    start=True,
                                    stop=True,
                                )
                            nc.scalar.activation(
                                ptg[:, ml, :],
                                ps,
                                AF.Exp,
                                bias=biasT[:, bl * 8 + m : bl * 8 + m + 1],
                                scale=0.125,
                            )
                            nc.vector.tensor_mul(
                                ptg[:, ml, :], ptg[:, ml, :], b['maskT'][:, m, :]
                            )
                        for ml in range(4):
                            m = mg * 4 + ml
                            for n in range(2):
                                nc.tensor.matmul(
                                    pav[n],
                                    b['vh'][:, m, h, :],
                                    ptg[:, ml, n * 512 : (n + 1) * 512],
                                    start=(m == 0),
                                    stop=(m == 7),
                                )

                    avs = rsp.tile([65, 1024], F32, tag="avs", name=f"avs{bl}{h}")
                    for n in range(2):
                        nc.vector.tensor_copy(
                            avs[:, n * 512 : (n + 1) * 512], pav[n]
                        )

                    # transpose av result per lq-tile (4 tiles per PSUM group):
                    # [65, 128] -> [128, 65]; col 64 holds softmax row-sums
                    for tg4 in range(2):
                        ptx = psC.tile([128, 4, 65], F32, tag="tr",
                                       name=f"ptx{bl}{h}_{tg4}")
                        for tt in range(4):
                            t = tg4 * 4 + tt
                            nc.tensor.transpose(
                                ptx[:, tt, :],
                                avs[:, t * 128 : (t + 1) * 128],
                                identf[0:65, 0:65],
                            )
                        nc.vector.reciprocal(
                            recip[:, tg4 * 4 : (tg4 + 1) * 4],
                            ptx[:, :, 64],
                        )
                        for tt in range(4):
                            t = tg4 * 4 + tt
                            nc.vector.tensor_scalar(
                                b['xn'][:, t, h * 64 : (h + 1) * 64],
                                ptx[:, tt, 0:64],
                                recip[:, t : t + 1],
                                None,
                                OP.mult,
                            )

                    # transpose P^T back (8 bf16 tiles pack into one PSUM
                    # bank), normalize with the row-sum reciprocal during the
                    # PSUM->SBUF copy, and flush to HBM
                    for t in range(8):
                        pstB = psC.tile([128, 1024], BF16, tag="tr",
                                        padded_shape=[128, 1024],
                                        name=f"pst{bl}{h}_{t}")
                        for i in range(8):
                            nc.tensor.transpose(
                                pstB[:, i * 128 : (i + 1) * 128],
                                ptgs[i // 4][:, i % 4, t * 128 : (t + 1) * 128],
                                identb,
                            )
                        st = stg.tile([128, 1024], BF16, tag="stage",
                                      name=f"st{bl}{h}_{t}")
                        if t % 2 == 0:
                            nc.vector.tensor_scalar(
                                st, pstB, recip[:, t : t + 1], None, OP.mult
                            )
                        else:
                            nc.scalar.mul(st, pstB, recip[:, t : t + 1])
                        nc.gpsimd.dma_start(
                            attn_d[h, bl, t * 128 : (t + 1) * 128, :], st
                        )

            def emit_xretr(bl, j, grp):
                b = bt[bl]
                pst = psA.tile([128, 512], BF16, tag="s",
                               padded_shape=[128, 512], name=f"xr{bl}_{j}_{grp}")
                for tt in range(4):
                    t = grp * 4 + tt
                    nc.tensor.transpose(
                        pst[:, tt * 128 : (tt + 1) * 128],
                        b['xn'][:, t, j * 128 : (j + 1) * 128],
                        identb,
                    )
                nc.scalar.copy(b['outT'][:, j, grp * 512 : (grp + 1) * 512], pst)

            def emit_fc_tile(bl, t):
                b = bt[bl]
                psf = psC.tile([128, 512], F32, tag="tr", name=f"psf{bl}_{t}")
                for j in range(4):
                    nc.tensor.matmul(
                        psf,
                        b['outT'][:, j, t * 128 : (t + 1) * 128],
                        fcw[:, j, :],
                        start=(j == 0),
                        stop=False,
                    )
                nc.tensor.matmul(psf, onesrowb, fcb, start=False, stop=True)

                fco = scr.tile([128, 512], F32, tag="fco", name=f"fco{bl}_{t}")
                nc.scalar.copy(fco, psf)
                sum_t = sp.tile([128, 1], F32, tag="ln", name=f"s{bl}_{t}")
                nc.vector.tensor_reduce(sum_t, psf, axis=AX.X, op=OP.add)
                sumsq = sp.tile([128, 1], F32, tag="ln", name=f"q{bl}_{t}")
                nc.scalar.activation(psf, psf, AF.Square, accum_out=sumsq)
                mean_t = sp.tile([128, 1], F32, tag="ln", name=f"m{bl}_{t}")
                nc.vector.tensor_scalar(mean_t, sum_t, 1.0 / 512.0, None, OP.mult)
                m2 = sp.tile([128, 1], F32, tag="ln", name=f"m2{bl}_{t}")
                nc.vector.tensor_mul(m2, mean_t, mean_t)
                var_t = sp.tile([128, 1], F32, tag="ln", name=f"v{bl}_{t}")
                nc.vector.scalar_tensor_tensor(
                    var_t, sumsq, 1.0 / 512.0, m2, OP.mult, OP.subtract
                )
                t1 = sp.tile([128, 1], F32, tag="ln", name=f"t{bl}_{t}")
                nc.scalar.activation(t1, var_t, AF.Ln, bias=eps, scale=1.0)
                rstd = sp.tile([128, 1], F32, tag="ln", name=f"r{bl}_{t}")
                nc.scalar.activation(rstd, t1, AF.Exp, scale=-0.5)
                y1 = scr.tile([128, 512], F32, tag="y1", name=f"y1{bl}_{t}")
                nc.vector.scalar_tensor_tensor(
                    y1, fco, mean_t, lng_b, OP.subtract, OP.mult
                )
                y2 = scr.tile([128, 512], F32, tag="y1", name=f"y2{bl}_{t}")
                nc.vector.scalar_tensor_tensor(
                    y2, y1, rstd, lnb_b, OP.mult, OP.add
                )
                nc.sync.dma_start(out_d[bl, t * 128 : (t + 1) * 128, :], y2)

            def emit_fc(bl):
                for j in range(4):
                    for grp in range(2):
                        emit_xretr(bl, j, grp)
                for t in range(8):
                    emit_fc_tile(bl, t)

            emit_proj(0)
            emit_heads(0)
            emit_proj(1)

            emit_fc(0)
            emit_heads(1)
            emit_fc(1)

    nc.compile()
    return nc


def kernel(**inputs):
    inp = {k: np.asarray(v) for k, v in inputs.items()}
    q, k, v = inp["q"], inp["k"], inp["v"]
    mask = inp["mask"]
    td = inp["time_diff"].astype(np.float32)

    qT = np.ascontiguousarray(q.astype(np.float32).transpose(0, 2, 1))
    kT = np.ascontiguousarray(k.astype(np.float32).transpose(0, 2, 1))
    import ml_dtypes as _mld
    vT = np.ascontiguousarray(v.transpose(0, 2, 1).astype(_mld.bfloat16))
    import ml_dtypes
    maskT = np.ascontiguousarray(
        (~mask).transpose(0, 2, 1).astype(ml_dtypes.bfloat16)
    )
    tgrid = np.ascontiguousarray(td.reshape(128, 128))

    common = {
        "tp": inp["time_plus"].astype(np.float32).reshape(1, 1),
        "tm": inp["time_mul"].astype(np.float32).reshape(1, 1),
        "wqT": np.ascontiguousarray(inp["Wq"].astype(np.float32).T),
        "wkT": np.ascontiguousarray(inp["Wk"].astype(np.float32).T),
        "wvT": np.ascontiguousarray(inp["Wv"].T.astype(_mld.bfloat16)),
        "fcwT": np.ascontiguousarray(inp["fc_w"].T.astype(ml_dtypes.bfloat16)),
        "fcb": inp["fc_b"].astype(ml_dtypes.bfloat16).reshape(1, D),
        "lng": inp["ln_g"].astype(_mld.bfloat16).reshape(1, D),
        "lnb": inp["ln_b"].astype(_mld.bfloat16).reshape(1, D),
        "ident": np.eye(128, dtype=np.float32),
        "identb": np.eye(128, dtype=ml_dtypes.bfloat16),
        "onesrow": np.ones((1, 128), dtype=np.float32),
        "onesrowb": np.ones((1, 128), dtype=ml_dtypes.bfloat16),
    }

    if "nc" not in _cache:
        _cache["nc"] = _build()
    nc = _cache["nc"]

    in_maps = []
    for c in range(NCORES):
        s = slice(c * BPC, (c + 1) * BPC)
        order = list(range(c * BPC, (c + 1) * BPC)) + [
            b for b in range(B) if not (c * BPC <= b < (c + 1) * BPC)
        ]
        in_maps.append(
            {
                "qT": qT[s],
                "kT": kT[s],
                "vT": vT[s],
                "maskT": maskT[s],
                "tgrid": np.ascontiguousarray(td[order].reshape(128, 128)),
                **common,
            }
        )

    res = bass_utils.run_bass_kernel_spmd(nc, in_maps, core_ids=list(range(NCORES)))

    out = np.empty((B, L, D), np.float32)
    attn = np.empty((H, B, L, L), np.float32)
    for c in range(NCORES):
        r = res.results[c]
        out[c * BPC : (c + 1) * BPC] = r["out_o"]
        attn[:, c * BPC : (c + 1) * BPC] = r["attn_o"]
    return out, attn.reshape(H * B, L, L)


# revision 35
# speedup vs baseline: 1.0354x; 1.0020x over previous
"""Trainium2 Bass kernel for nn_MultiHeadAttention (B16 L1024 D512 H8).

Sharding: pure data-parallel, 2 batches per core across 8 NeuronCores.

Per-core device pipeline (per batch, per head):
  - QKV projections as f32r matmuls producing transposed layouts
    (qhT/khT = [dk, lq]; vh = [lk, dv] with an appended ones column).
  - Scores computed transposed: S^T[lk, lq] = khT.T @ qhT so the per-key
    time-decay bias is a per-partition ACT bias fused into the exp, and
    P^T feeds the P@V matmul directly with no transpose.
  - Mask applied on the f32 PSUM scores via copy_predicated(-1e30).
  - P@V uses vh with a ones column; the extra output row gives softmax
    row-sums for free.
  - Attention probabilities are transposed back per 128x128 tile on the
    PE, normalized during the PSUM->SBUF copy (per-partition reciprocal),
    and DMA'd out.
  - fc projection consumes the P@V output layout directly; fc bias added
    via a K=1 ones matmul; LayerNorm rstd = exp(-0.5*ln(var+eps)) so the
    whole kernel uses one ACT table set (natural_log_exp).
"""

import sys

sys.path.insert(0, "/opt/trn_rl_repo")

import numpy as np

import concourse.bass as bass
import concourse.bacc as bacc
import concourse.mybir as mybir
import concourse.tile as tile
from concourse import bass_utils

B, L, D, H, DK, DV = 16, 1024, 512, 8, 64, 64
LN_EPS = 1e-5
E = float(np.e)
NCORES = 8
BPC = B // NCORES  # batches per core

F32 = mybir.dt.float32
F32R = mybir.dt.float32r
U8 = mybir.dt.uint8
BF16 = mybir.dt.bfloat16
AF = mybir.ActivationFunctionType
OP = mybir.AluOpType
AX = mybir.AxisListType

_cache = {}


def _patch_act_tables():
    """Force every activation func onto the natural_log_exp set so the
    kernel needs exactly one ACT table load (Exp+Ln+Copy+Square all live
    there); the default chooser thrashes between exp-only and ln-only sets."""
    from concourse.hw_specs import get_activation_tables as _gat

    def single(arch):
        t = _gat(arch)
        return {
            k: (v if k == "natural_log_exp_and_others" else type(v)())
            for k, v in t.items()
        }

    bacc.get_activation_tables = single


def _build():
    _patch_act_tables()
    nc = bacc.Bacc("TRN2", target_bir_lowering=False, debug=False)

    dt_in = {}
    def din(name, shape, dt=F32):
        dt_in[name] = nc.dram_tensor(name, shape, dt, kind="ExternalInput").ap()
        return dt_in[name]

    qT_d = din("qT", (BPC, D, L))
    kT_d = din("kT", (BPC, D, L))
    vT_d = din("vT", (BPC, D, L), BF16)
    maskT_d = din("maskT", (BPC, L, L), BF16)
    tg_d = din("tgrid", (128, 128))
    tp_d = din("tp", (1, 1))
    tm_d = din("tm", (1, 1))
    wq_d = din("wqT", (D, H * DK))
    wk_d = din("wkT", (D, H * DK))
    wv_d = din("wvT", (D, H * DV), BF16)
    fcw_d = din("fcwT", (H * DV, D), BF16)
    fcb_d = din("fcb", (1, D), BF16)
    lng_d = din("lng", (1, D), BF16)
    lnb_d = din("lnb", (1, D), BF16)
    id_d = din("ident", (128, 128))
    idb_d = din("identb", (128, 128), BF16)
    onesr_d = din("onesrow", (1, 128))
    onesrb_d = din("onesrowb", (1, 128), BF16)

    out_d = nc.dram_tensor("out_o", (BPC, L, D), F32, kind="ExternalOutput").ap()
    attn_d = nc.dram_tensor("attn_o", (H, BPC, L, L), F32, kind="ExternalOutput").ap()

    with tile.TileContext(nc) as tc:
        with (
            tc.tile_pool(name="const", bufs=1) as cp,
            tc.tile_pool(name="small", bufs=8) as sp,
            tc.tile_pool(name="xt", bufs=2) as xp,
            tc.tile_pool(name="batch", bufs=1) as bp,
            tc.tile_pool(name="pt", bufs=4) as ptp,
            tc.tile_pool(name="stg", bufs=7) as stg,
            tc.tile_pool(name="rsp", bufs=2) as rsp,
            tc.tile_pool(name="scr", bufs=2) as scr,
            tc.tile_pool(name="psA", bufs=2, space="PSUM") as psA,
            tc.tile_pool(name="psB", bufs=2, space="PSUM") as psB,
            tc.tile_pool(name="psC", bufs=2, space="PSUM") as psC,
        ):
            # ---- constants ----
            identf = cp.tile([128, 128], F32)
            nc.scalar.dma_start(identf, id_d)
            onesrow = cp.tile([1, 128], F32)
            nc.scalar.dma_start(onesrow, onesr_d)
            onescol = cp.tile([128, 1], F32)
            nc.scalar.dma_start(onescol, onesr_d.rearrange("a b -> b a"))
            tg = cp.tile([128, 128], F32)
            nc.scalar.dma_start(tg, tg_d)
            tp_sb = sp.tile([1, 1], F32)
            nc.scalar.dma_start(tp_sb, tp_d)
            tm_sb = sp.tile([1, 1], F32)
            nc.scalar.dma_start(tm_sb, tm_d)
            wq = cp.tile([128, 4, 512], F32R)
            wk = cp.tile([128, 4, 512], F32R)
            for t, d in ((wq, wq_d), (wk, wk_d)):
                nc.scalar.dma_start(t, d.rearrange("(c p) n -> p c n", p=128).bitcast(F32R))
            wv = cp.tile([128, 4, 512], BF16)
            nc.scalar.dma_start(wv, wv_d.rearrange("(c p) n -> p c n", p=128))
            fcw = cp.tile([128, 4, 512], BF16)
            nc.scalar.dma_start(fcw, fcw_d.rearrange("(c p) n -> p c n", p=128))
            identb = cp.tile([128, 128], BF16)
            nc.scalar.dma_start(identb, idb_d)
            onesrowb = cp.tile([1, 128], BF16)
            nc.scalar.dma_start(onesrowb, onesrb_d)
            fcb = cp.tile([1, 512], BF16)
            nc.scalar.dma_start(fcb, fcb_d)
            lng_b = cp.tile([128, 512], BF16)
            lnb_b = cp.tile([128, 512], BF16)
            for t, d in ((lng_b, lng_d), (lnb_b, lnb_d)):
                nc.scalar.dma_start(
                    t, bass.AP(tensor=d.tensor, offset=d.offset, ap=[[0, 128]] + d.ap[1:])
                )
            eps = cp.tile([128, 1], F32)
            nc.vector.memset(eps, LN_EPS)

            # ---- phase A: time-decay bias grid ----
            rs = sp.tile([128, 1], F32)
            nc.vector.tensor_reduce(rs, tg, axis=AX.X, op=OP.add)
            rmx = sp.tile([128, 1], F32)
            nc.vector.tensor_reduce(rmx, tg, axis=AX.X, op=OP.max)
            pstot = psC.tile([1, 1], F32, tag="tr")
            nc.tensor.matmul(pstot, rs, onescol, start=True, stop=True)
            tot = sp.tile([1, 1], F32)
            nc.vector.tensor_copy(tot, pstot)
            psmx = psC.tile([1, 128], F32, tag="tr")
            nc.tensor.transpose(psmx, rmx, identf)
            mx1 = sp.tile([1, 1], F32)
            nc.vector.tensor_reduce(mx1, psmx, axis=AX.X, op=OP.max)
            rtot = sp.tile([1, 1], F32)
            nc.vector.reciprocal(rtot, tot)
            invmean = sp.tile([1, 1], F32)
            nc.vector.tensor_scalar(invmean, rtot, float(B * L), None, OP.mult)
            tdmax = sp.tile([1, 1], F32)
            nc.vector.tensor_mul(tdmax, mx1, invmean)
            # softplus(tp) = ln(1 + exp(tp))
            e1 = sp.tile([1, 1], F32)
            nc.scalar.activation(e1, tp_sb, AF.Exp)
            e2 = sp.tile([1, 1], F32)
            nc.vector.tensor_scalar(e2, e1, 1.0, None, OP.add)
            spl = sp.tile([1, 1], F32)
            nc.scalar.activation(spl, e2, AF.Ln)
            ace = sp.tile([1, 1], F32)
            nc.vector.tensor_mul(ace, spl, tdmax)
            nc.vector.tensor_scalar(ace, ace, E, None, OP.add)

            def bcast128(src):
                ps = psC.tile([128, 1], F32, tag="tr")
                nc.tensor.matmul(ps, onesrow, src, start=True, stop=True)
                dst = sp.tile([128, 1], F32)
                nc.scalar.copy(dst, ps)
                return dst

            invmean_b = bcast128(invmean)
            ace_b = bcast128(ace)
            tm_b = bcast128(tm_sb)

            lnu = cp.tile([128, 128], F32)
            nc.scalar.activation(lnu, tg, AF.Ln, bias=ace_b, scale=invmean_b)
            binv = cp.tile([128, 128], F32)
            nc.vector.reciprocal(binv, lnu)
            bgrid = cp.tile([128, 128], F32)
            nc.vector.tensor_scalar(bgrid, binv, tm_b, None, OP.mult)
            psbt = psC.tile([128, 128], F32, tag="tr")
            nc.tensor.transpose(psbt, bgrid, identf)
            biasT = cp.tile([128, 128], F32)
            nc.scalar.copy(biasT, psbt)

            # ---- per-batch, software-pipelined phases ----
            bt = {}

            def emit_proj(bl):
                qhT = bp.tile([128, 4, 1024], F32R, tag="qhT", name=f"qhT{bl}")
                khT = bp.tile([128, 4, 1024], F32R, tag="khT", name=f"khT{bl}")
                vh = bp.tile([128, 8, 8, 65], BF16, tag="vh", name=f"vh{bl}")
                maskT = bp.tile([128, 8, 1024], BF16, tag="maskT", name=f"maskT{bl}")
                outT = bp.tile([128, 4, 1024], BF16, tag="outT", name=f"outT{bl}")
                xn = bp.tile([128, 8, 512], BF16, tag="xn", bufs=2, name=f"xn{bl}")
                bt[bl] = dict(qhT=qhT, khT=khT, vh=vh, maskT=maskT, outT=outT, xn=xn)
                mask_r = maskT_d[bl].rearrange("(m p) l -> p m l", p=128)
                for m in range(8):
                    nc.sync.dma_start(maskT[:, m, :], mask_r[:, m, :])

                for src_d, w_sb, dst in ((qT_d, wq, qhT),):
                    xin = xp.tile([128, 4, 1024], F32R, tag="xt", name=f"x{bl}_{dst.tensor.name}")
                    src_r = src_d[bl].rearrange("(c p) l -> p c l", p=128).bitcast(F32R)
                    for kc in range(4):
                        nc.sync.dma_start(xin[:, kc, :], src_r[:, kc, :])
                    for m in range(4):
                        ps = psA.tile([128, 1024], F32, tag="s")
                        for n in range(2):
                            for kc in range(4):
                                nc.tensor.matmul(
                                    ps[:, n * 512 : (n + 1) * 512],
                                    w_sb[:, kc, m * 128 : (m + 1) * 128],
                                    xin[:, kc, n * 512 : (n + 1) * 512],
                                    start=(kc == 0),
                                    stop=(kc == 3),
                                )
                        nc.vector.tensor_scalar(
                            dst[:, m, :], ps, 1.0, None, OP.mult
                        )

                xin = xp.tile([128, 4, 1024], F32R, tag="xt", name=f"x{bl}_k")
                src_r = kT_d[bl].rearrange("(c p) l -> p c l", p=128).bitcast(F32R)
                for kc in range(4):
                    nc.sync.dma_start(xin[:, kc, :], src_r[:, kc, :])
                for m in range(4):
                    for n in range(2):
                        ps = psB.tile([128, 512], F32, tag="av")
                        for kc in range(4):
                            nc.tensor.matmul(
                                ps,
                                wk[:, kc, m * 128 : (m + 1) * 128],
                                xin[:, kc, n * 512 : (n + 1) * 512],
                                start=(kc == 0),
                                stop=(kc == 3),
                            )
                        nc.vector.tensor_scalar(
                            khT[:, m, n * 512 : (n + 1) * 512], ps, 1.0,
                            None, OP.mult,
                        )

                xin = xp.tile([128, 4, 1024], BF16, tag="xt", name=f"x{bl}_v")
                src_r = vT_d[bl].rearrange("(c p) l -> p c l", p=128)
                for kc in range(4):
                    nc.sync.dma_start(xin[:, kc, :], src_r[:, kc, :])
                for m in range(8):
                    ps = psB.tile([128, 512], F32, tag="av")
                    for kc in range(4):
                        nc.tensor.matmul(
                            ps,
                            xin[:, kc, m * 128 : (m + 1) * 128],
                            wv[:, kc, :],
                            start=(kc == 0),
                            stop=(kc == 3),
                        )
                    # scatter heads into [h, 65] layout (col 64 = ones)
                    nc.vector.tensor_scalar(
                        vh[:, m, :, 0:64],
                        ps.rearrange("p (h e) -> p h e", h=8),
                        1.0,
                        None,
                        OP.mult,
                    )
                    nc.vector.memset(vh[:, m, :, 64:65], 1.0)

            def emit_heads(bl, post=None):
                b = bt[bl]
                for h in range(8):
                    if post is not None:
                        post(h)
                    po = (h % 2) * 64
                    ch = h // 2
                    qh = b['qhT'][po : po + 64, ch, :]
                    kh = b['khT'][po : po + 64, ch, :]

                    recip = rsp.tile([128, 8], F32, tag="recip", name=f"rc{bl}{h}")
                    pav = [
                        psB.tile([65, 512], F32, tag="av", name=f"pav{bl}{h}_{_n}")
                        for _n in range(2)
                    ]

                    ptgs = []
                    for mg in range(2):
                        ptg = ptp.tile([128, 4, 1024], BF16, tag="pt",
                                       name=f"ptg{bl}{h}_{mg}")
                        ptgs.append(ptg)
                        for ml in range(4):
                            m = mg * 4 + ml
                            ps = psA.tile([128, 1024], F32, tag="s")
                            for n in range(2):
                                nc.tensor.matmul(
                                    ps[:, n * 512 : (n + 1) * 512],
                                    kh[:, m * 128 : (m + 1) * 128],
                                    qh[:, n * 512 : (n + 1) * 512],
                                    start=True,
                                    stop=True,
                                )
                            nc.scalar.activation(
                                ptg[:, ml, :],
                                ps,
                                AF.Exp,
                                bias=biasT[:, bl * 8 + m : bl * 8 + m + 1],
                                scale=0.125,
                            )
                            nc.vector.tensor_mul(
                                ptg[:, ml, :], ptg[:, ml, :], b['maskT'][:, m, :]
                            )
                        for ml in range(4):
                            m = mg * 4 + ml
                            for n in range(2):
                                nc.tensor.matmul(
                                    pav[n],
                                    b['vh'][:, m, h, :],
                                    ptg[:, ml, n * 512 : (n + 1) * 512],
                                    start=(m == 0),
                                    stop=(m == 7),
                                )

                    avs = rsp.tile([65, 1024], F32, tag="avs", name=f"avs{bl}{h}")
                    for n in range(2):
                        nc.vector.tensor_copy(
                            avs[:, n * 512 : (n + 1) * 512], pav[n]
                        )

                    # transpose av result per lq-tile (4 tiles per PSUM group):
                    # [65, 128] -> [128, 65]; col 64 holds softmax row-sums
                    for tg4 in range(2):
                        ptx = psC.tile([128, 4, 65], F32, tag="tr",
                                       name=f"ptx{bl}{h}_{tg4}")
                        for tt in range(4):
                            t = tg4 * 4 + tt
                            nc.tensor.transpose(
                                ptx[:, tt, :],
                                avs[:, t * 128 : (t + 1) * 128],
                                identf[0:65, 0:65],
                            )
                        nc.vector.reciprocal(
                            recip[:, tg4 * 4 : (tg4 + 1) * 4],
                            ptx[:, :, 64],
                        )
                        for tt in range(4):
                            t = tg4 * 4 + tt
                            nc.vector.tensor_scalar(
                                b['xn'][:, t, h * 64 : (h + 1) * 64],
                                ptx[:, tt, 0:64],
                                recip[:, t : t + 1],
                                None,
                                OP.mult,
                            )

                    # transpose P^T back (8 bf16 tiles pack into one PSUM
                    # bank), normalize with the row-sum reciprocal during the
                    # PSUM->SBUF copy, and flush to HBM
                    for t in range(8):
                        pstB = psC.tile([128, 1024], BF16, tag="tr",
                                        padded_shape=[128, 1024],
                                        name=f"pst{bl}{h}_{t}")
                        for i in range(8):
                            nc.tensor.transpose(
                                pstB[:, i * 128 : (i + 1) * 128],
                                ptgs[i // 4][:, i % 4, t * 128 : (t + 1) * 128],
                                identb,
                            )
                        st = stg.tile([128, 1024], BF16, tag="stage",
                                      name=f"st{bl}{h}_{t}")
                        if t % 2 == 0:
                            nc.vector.tensor_scalar(
                                st, pstB, recip[:, t : t + 1], None, OP.mult
                            )
                        else:
                            nc.scalar.mul(st, pstB, recip[:, t : t + 1])
                        nc.gpsimd.dma_start(
                            attn_d[h, bl, t * 128 : (t + 1) * 128, :], st
                        )

            def emit_xretr(bl, j, grp):
                b = bt[bl]
                pst = psA.tile([128, 512], BF16, tag="s",
                               padded_shape=[128, 512], name=f"xr{bl}_{j}_{grp}")
                for tt in range(4):
                    t = grp * 4 + tt
                    nc.tensor.transpose(
                        pst[:, tt * 128 : (tt + 1) * 128],
                        b['xn'][:, t, j * 128 : (j + 1) * 128],
                        identb,
                    )
                nc.scalar.copy(b['outT'][:, j, grp * 512 : (grp + 1) * 512], pst)

            def emit_fc_tile(bl, t):
                b = bt[bl]
                psf = psC.tile([128, 512], F32, tag="tr", name=f"psf{bl}_{t}")
                for j in range(4):
                    nc.tensor.matmul(
                        psf,
                        b['outT'][:, j, t * 128 : (t + 1) * 128],
                        fcw[:, j, :],
                        start=(j == 0),
                        stop=False,
                    )
                nc.tensor.matmul(psf, onesrowb, fcb, start=False, stop=True)

                fco = scr.tile([128, 512], F32, tag="fco", bufs=1, name=f"fco{bl}_{t}")
                nc.scalar.copy(fco, psf)
                sum_t = sp.tile([128, 1], F32, tag="ln", name=f"s{bl}_{t}")
                nc.vector.tensor_reduce(sum_t, psf, axis=AX.X, op=OP.add)
                sumsq = sp.tile([128, 1], F32, tag="ln", name=f"q{bl}_{t}")
                nc.scalar.activation(psf, psf, AF.Square, accum_out=sumsq)
                mean_t = sp.tile([128, 1], F32, tag="ln", name=f"m{bl}_{t}")
                nc.vector.tensor_scalar(mean_t, sum_t, 1.0 / 512.0, None, OP.mult)
                m2 = sp.tile([128, 1], F32, tag="ln", name=f"m2{bl}_{t}")
                nc.vector.tensor_mul(m2, mean_t, mean_t)
                var_t = sp.tile([128, 1], F32, tag="ln", name=f"v{bl}_{t}")
                nc.vector.scalar_tensor_tensor(
                    var_t, sumsq, 1.0 / 512.0, m2, OP.mult, OP.subtract
                )
                t1 = sp.tile([128, 1], F32, tag="ln", name=f"t{bl}_{t}")
                nc.scalar.activation(t1, var_t, AF.Ln, bias=eps, scale=1.0)
                rstd = sp.tile([128, 1], F32, tag="ln", name=f"r{bl}_{t}")
                nc.scalar.activation(rstd, t1, AF.Exp, scale=-0.5)
                y1 = scr.tile([128, 512], F32, tag="y1", name=f"y1{bl}_{t}")
                nc.vector.scalar_tensor_tensor(
                    y1, fco, mean_t, lng_b, OP.subtract, OP.mult
                )
                y2 = scr.tile([128, 512], F32, tag="y1", name=f"y2{bl}_{t}")
                nc.vector.scalar_tensor_tensor(
                    y2, y1, rstd, lnb_b, OP.mult, OP.add
                )
                nc.sync.dma_start(out_d[bl, t * 128 : (t + 1) * 128, :], y2)

            def emit_fc(bl):
                for j in range(4):
                    for grp in range(2):
                        emit_xretr(bl, j, grp)
                for t in range(8):
                    emit_fc_tile(bl, t)

            emit_proj(0)
            emit_heads(0)
            emit_proj(1)

            emit_fc(0)
            emit_heads(1)
            emit_fc(1)

    nc.compile()
    return nc


def kernel(**inputs):
    inp = {k: np.asarray(v) for k, v in inputs.items()}
    q, k, v = inp["q"], inp["k"], inp["v"]
    mask = inp["mask"]
    td = inp["time_diff"].astype(np.float32)

    qT = np.ascontiguousarray(q.astype(np.float32).transpose(0, 2, 1))
    kT = np.ascontiguousarray(k.astype(np.float32).transpose(0, 2, 1))
    import ml_dtypes as _mld
    vT = np.ascontiguousarray(v.transpose(0, 2, 1).astype(_mld.bfloat16))
    import ml_dtypes
    maskT = np.ascontiguousarray(
        (~mask).transpose(0, 2, 1).astype(ml_dtypes.bfloat16)
    )
    tgrid = np.ascontiguousarray(td.reshape(128, 128))

    common = {
        "tp": inp["time_plus"].astype(np.float32).reshape(1, 1),
        "tm": inp["time_mul"].astype(np.float32).reshape(1, 1),
        "wqT": np.ascontiguousarray(inp["Wq"].astype(np.float32).T),
        "wkT": np.ascontiguousarray(inp["Wk"].astype(np.float32).T),
        "wvT": np.ascontiguousarray(inp["Wv"].T.astype(_mld.bfloat16)),
        "fcwT": np.ascontiguousarray(inp["fc_w"].T.astype(ml_dtypes.bfloat16)),
        "fcb": inp["fc_b"].astype(ml_dtypes.bfloat16).reshape(1, D),
        "lng": inp["ln_g"].astype(_mld.bfloat16).reshape(1, D),
        "lnb": inp["ln_b"].astype(_mld.bfloat16).reshape(1, D),
        "ident": np.eye(128, dtype=np.float32),
        "identb": np.eye(128, dtype=ml_dtypes.bfloat16),
        "onesrow": np.ones((1, 128), dtype=np.float32),
        "onesrowb": np.ones((1, 128), dtype=ml_dtypes.bfloat16),
    }

    if "nc" not in _cache:
        _cache["nc"] = _build()
    nc = _cache["nc"]

    in_maps = []
    for c in range(NCORES):
        s = slice(c * BPC, (c + 1) * BPC)
        order = list(range(c * BPC, (c + 1) * BPC)) + [
            b for b in range(B) if not (c * BPC <= b < (c + 1) * BPC)
        ]
        in_maps.append(
            {
                "qT": qT[s],
                "kT": kT[s],
                "vT": vT[s],
                "maskT": maskT[s],
                "tgrid": np.ascontiguousarray(td[order].reshape(128, 128)),
                **common,
            }
        )

    res = bass_utils.run_bass_kernel_spmd(nc, in_maps, core_ids=list(range(NCORES)))

    out = np.empty((B, L, D), np.float32)
    attn = np.empty((H, B, L, L), np.float32)
    for c in range(NCORES):
        r = res.results[c]
        out[c * BPC : (c + 1) * BPC] = r["out_o"]
        attn[:, c * BPC : (c + 1) * BPC] = r["attn_o"]
    return out, attn.reshape(H * B, L, L)


# revision 36
# speedup vs baseline: 1.0619x; 1.0256x over previous
"""Trainium2 Bass kernel for nn_MultiHeadAttention (B16 L1024 D512 H8).

Sharding: pure data-parallel, 2 batches per core across 8 NeuronCores.

Per-core device pipeline (per batch, per head):
  - QKV projections as f32r matmuls producing transposed layouts
    (qhT/khT = [dk, lq]; vh = [lk, dv] with an appended ones column).
  - Scores computed transposed: S^T[lk, lq] = khT.T @ qhT so the per-key
    time-decay bias is a per-partition ACT bias fused into the exp, and
    P^T feeds the P@V matmul directly with no transpose.
  - Mask applied on the f32 PSUM scores via copy_predicated(-1e30).
  - P@V uses vh with a ones column; the extra output row gives softmax
    row-sums for free.
  - Attention probabilities are transposed back per 128x128 tile on the
    PE, normalized during the PSUM->SBUF copy (per-partition reciprocal),
    and DMA'd out.
  - fc projection consumes the P@V output layout directly; fc bias added
    via a K=1 ones matmul; LayerNorm rstd = exp(-0.5*ln(var+eps)) so the
    whole kernel uses one ACT table set (natural_log_exp).
"""

import sys

sys.path.insert(0, "/opt/trn_rl_repo")

import numpy as np

import concourse.bass as bass
import concourse.bacc as bacc
import concourse.mybir as mybir
import concourse.tile as tile
from concourse import bass_utils

B, L, D, H, DK, DV = 16, 1024, 512, 8, 64, 64
LN_EPS = 1e-5
E = float(np.e)
NCORES = 8
BPC = B // NCORES  # batches per core

F32 = mybir.dt.float32
F32R = mybir.dt.float32r
U8 = mybir.dt.uint8
BF16 = mybir.dt.bfloat16
AF = mybir.ActivationFunctionType
OP = mybir.AluOpType
AX = mybir.AxisListType

_cache = {}


def _patch_act_tables():
    """Force every activation func onto the natural_log_exp set so the
    kernel needs exactly one ACT table load (Exp+Ln+Copy+Square all live
    there); the default chooser thrashes between exp-only and ln-only sets."""
    from concourse.hw_specs import get_activation_tables as _gat

    def single(arch):
        t = _gat(arch)
        return {
            k: (v if k == "natural_log_exp_and_others" else type(v)())
            for k, v in t.items()
        }

    bacc.get_activation_tables = single


def _build():
    _patch_act_tables()
    nc = bacc.Bacc("TRN2", target_bir_lowering=False, debug=False)

    dt_in = {}
    def din(name, shape, dt=F32):
        dt_in[name] = nc.dram_tensor(name, shape, dt, kind="ExternalInput").ap()
        return dt_in[name]

    qT_d = din("qT", (BPC, D, L))
    kT_d = din("kT", (BPC, D, L))
    vT_d = din("vT", (BPC, D, L), BF16)
    maskT_d = din("maskT", (BPC, L, L), BF16)
    tg_d = din("tgrid", (128, 128))
    tp_d = din("tp", (1, 1))
    tm_d = din("tm", (1, 1))
    wq_d = din("wqT", (D, H * DK))
    wk_d = din("wkT", (D, H * DK))
    wv_d = din("wvT", (D, H * DV), BF16)
    fcw_d = din("fcwT", (H * DV, D), BF16)
    fcb_d = din("fcb", (1, D), BF16)
    lng_d = din("lng", (1, D), BF16)
    lnb_d = din("lnb", (1, D), BF16)
    id_d = din("ident", (128, 128))
    idb_d = din("identb", (128, 128), BF16)
    onesr_d = din("onesrow", (1, 128))
    onesrb_d = din("onesrowb", (1, 128), BF16)

    out_d = nc.dram_tensor("out_o", (BPC, L, D), F32, kind="ExternalOutput").ap()
    attn_d = nc.dram_tensor("attn_o", (H, BPC, L, L), F32, kind="ExternalOutput").ap()

    with tile.TileContext(nc) as tc:
        with (
            tc.tile_pool(name="const", bufs=1) as cp,
            tc.tile_pool(name="small", bufs=8) as sp,
            tc.tile_pool(name="xt", bufs=2) as xp,
            tc.tile_pool(name="batch", bufs=1) as bp,
            tc.tile_pool(name="pt", bufs=4) as ptp,
            tc.tile_pool(name="stg", bufs=7) as stg,
            tc.tile_pool(name="rsp", bufs=2) as rsp,
            tc.tile_pool(name="scr", bufs=2) as scr,
            tc.tile_pool(name="psA", bufs=2, space="PSUM") as psA,
            tc.tile_pool(name="psB", bufs=2, space="PSUM") as psB,
            tc.tile_pool(name="psC", bufs=2, space="PSUM") as psC,
        ):
            # ---- constants ----
            identf = cp.tile([128, 128], F32)
            nc.scalar.dma_start(identf, id_d)
            onesrow = cp.tile([1, 128], F32)
            nc.scalar.dma_start(onesrow, onesr_d)
            onescol = cp.tile([128, 1], F32)
            nc.scalar.dma_start(onescol, onesr_d.rearrange("a b -> b a"))
            tg = cp.tile([128, 128], F32)
            nc.scalar.dma_start(tg, tg_d)
            tp_sb = sp.tile([1, 1], F32)
            nc.scalar.dma_start(tp_sb, tp_d)
            tm_sb = sp.tile([1, 1], F32)
            nc.scalar.dma_start(tm_sb, tm_d)
            wq = cp.tile([128, 4, 512], F32R)
            wk = cp.tile([128, 4, 512], F32R)
            for t, d in ((wq, wq_d), (wk, wk_d)):
                nc.scalar.dma_start(t, d.rearrange("(c p) n -> p c n", p=128).bitcast(F32R))
            wv = cp.tile([128, 4, 512], BF16)
            nc.scalar.dma_start(wv, wv_d.rearrange("(c p) n -> p c n", p=128))
            fcw = cp.tile([128, 4, 512], BF16)
            nc.scalar.dma_start(fcw, fcw_d.rearrange("(c p) n -> p c n", p=128))
            identb = cp.tile([128, 128], BF16)
            nc.scalar.dma_start(identb, idb_d)
            onesrowb = cp.tile([1, 128], BF16)
            nc.scalar.dma_start(onesrowb, onesrb_d)
            fcb = cp.tile([1, 512], BF16)
            nc.scalar.dma_start(fcb, fcb_d)
            lng_b = cp.tile([128, 512], BF16)
            lnb_b = cp.tile([128, 512], BF16)
            for t, d in ((lng_b, lng_d), (lnb_b, lnb_d)):
                nc.scalar.dma_start(
                    t, bass.AP(tensor=d.tensor, offset=d.offset, ap=[[0, 128]] + d.ap[1:])
                )
            eps = cp.tile([128, 1], F32)
            nc.vector.memset(eps, LN_EPS)

            # ---- phase A: time-decay bias grid ----
            rs = sp.tile([128, 1], F32)
            nc.vector.tensor_reduce(rs, tg, axis=AX.X, op=OP.add)
            rmx = sp.tile([128, 1], F32)
            nc.vector.tensor_reduce(rmx, tg, axis=AX.X, op=OP.max)
            pstot = psC.tile([1, 1], F32, tag="tr")
            nc.tensor.matmul(pstot, rs, onescol, start=True, stop=True)
            tot = sp.tile([1, 1], F32)
            nc.vector.tensor_copy(tot, pstot)
            psmx = psC.tile([1, 128], F32, tag="tr")
            nc.tensor.transpose(psmx, rmx, identf)
            mx1 = sp.tile([1, 1], F32)
            nc.vector.tensor_reduce(mx1, psmx, axis=AX.X, op=OP.max)
            rtot = sp.tile([1, 1], F32)
            nc.vector.reciprocal(rtot, tot)
            invmean = sp.tile([1, 1], F32)
            nc.vector.tensor_scalar(invmean, rtot, float(B * L), None, OP.mult)
            tdmax = sp.tile([1, 1], F32)
            nc.vector.tensor_mul(tdmax, mx1, invmean)
            # softplus(tp) = ln(1 + exp(tp))
            e1 = sp.tile([1, 1], F32)
            nc.scalar.activation(e1, tp_sb, AF.Exp)
            e2 = sp.tile([1, 1], F32)
            nc.vector.tensor_scalar(e2, e1, 1.0, None, OP.add)
            spl = sp.tile([1, 1], F32)
            nc.scalar.activation(spl, e2, AF.Ln)
            ace = sp.tile([1, 1], F32)
            nc.vector.tensor_mul(ace, spl, tdmax)
            nc.vector.tensor_scalar(ace, ace, E, None, OP.add)

            def bcast128(src):
                ps = psC.tile([128, 1], F32, tag="tr")
                nc.tensor.matmul(ps, onesrow, src, start=True, stop=True)
                dst = sp.tile([128, 1], F32)
                nc.scalar.copy(dst, ps)
                return dst

            invmean_b = bcast128(invmean)
            ace_b = bcast128(ace)
            tm_b = bcast128(tm_sb)

            lnu = cp.tile([128, 128], F32)
            nc.scalar.activation(lnu, tg, AF.Ln, bias=ace_b, scale=invmean_b)
            binv = cp.tile([128, 128], F32)
            nc.vector.reciprocal(binv, lnu)
            bgrid = cp.tile([128, 128], F32)
            nc.vector.tensor_scalar(bgrid, binv, tm_b, None, OP.mult)
            psbt = psC.tile([128, 128], F32, tag="tr")
            nc.tensor.transpose(psbt, bgrid, identf)
            biasT = cp.tile([128, 128], F32)
            nc.scalar.copy(biasT, psbt)

            # ---- per-batch, software-pipelined phases ----
            bt = {}

            def emit_proj(bl):
                qhT = bp.tile([128, 4, 1024], F32R, tag="qhT", name=f"qhT{bl}")
                khT = bp.tile([128, 4, 1024], F32R, tag="khT", name=f"khT{bl}")
                vh = bp.tile([128, 8, 8, 65], BF16, tag="vh", name=f"vh{bl}")
                maskT = bp.tile([128, 8, 1024], BF16, tag="maskT", name=f"maskT{bl}")
                outT = bp.tile([128, 4, 1024], BF16, tag="outT", name=f"outT{bl}")
                xn = bp.tile([128, 8, 512], BF16, tag="xn", bufs=2, name=f"xn{bl}")
                bt[bl] = dict(qhT=qhT, khT=khT, vh=vh, maskT=maskT, outT=outT, xn=xn)
                mask_r = maskT_d[bl].rearrange("(m p) l -> p m l", p=128)
                for m in range(8):
                    nc.sync.dma_start(maskT[:, m, :], mask_r[:, m, :])

                for src_d, w_sb, dst in ((qT_d, wq, qhT),):
                    xin = xp.tile([128, 4, 1024], F32R, tag="xt", name=f"x{bl}_{dst.tensor.name}")
                    src_r = src_d[bl].rearrange("(c p) l -> p c l", p=128).bitcast(F32R)
                    for kc in range(4):
                        nc.sync.dma_start(xin[:, kc, :], src_r[:, kc, :])
                    for m in range(4):
                        ps = psA.tile([128, 1024], F32, tag="s")
                        for n in range(2):
                            for kc in range(4):
                                nc.tensor.matmul(
                                    ps[:, n * 512 : (n + 1) * 512],
                                    w_sb[:, kc, m * 128 : (m + 1) * 128],
                                    xin[:, kc, n * 512 : (n + 1) * 512],
                                    start=(kc == 0),
                                    stop=(kc == 3),
                                )
                        nc.vector.tensor_scalar(
                            dst[:, m, :], ps, 1.0, None, OP.mult
                        )

                xin = xp.tile([128, 4, 1024], F32R, tag="xt", name=f"x{bl}_k")
                src_r = kT_d[bl].rearrange("(c p) l -> p c l", p=128).bitcast(F32R)
                for kc in range(4):
                    nc.sync.dma_start(xin[:, kc, :], src_r[:, kc, :])
                for m in range(4):
                    for n in range(2):
                        ps = psB.tile([128, 512], F32, tag="av")
                        for kc in range(4):
                            nc.tensor.matmul(
                                ps,
                                wk[:, kc, m * 128 : (m + 1) * 128],
                                xin[:, kc, n * 512 : (n + 1) * 512],
                                start=(kc == 0),
                                stop=(kc == 3),
                            )
                        nc.vector.tensor_scalar(
                            khT[:, m, n * 512 : (n + 1) * 512], ps, 1.0,
                            None, OP.mult,
                        )

                xin = xp.tile([128, 4, 1024], BF16, tag="xt", name=f"x{bl}_v")
                src_r = vT_d[bl].rearrange("(c p) l -> p c l", p=128)
                for kc in range(4):
                    nc.sync.dma_start(xin[:, kc, :], src_r[:, kc, :])
                for m in range(8):
                    ps = psB.tile([128, 512], F32, tag="av")
                    for kc in range(4):
                        nc.tensor.matmul(
                            ps,
                            xin[:, kc, m * 128 : (m + 1) * 128],
                            wv[:, kc, :],
                            start=(kc == 0),
                            stop=(kc == 3),
                        )
                    # scatter heads into [h, 65] layout (col 64 = ones)
                    nc.vector.tensor_scalar(
                        vh[:, m, :, 0:64],
                        ps.rearrange("p (h e) -> p h e", h=8),
                        1.0,
                        None,
                        OP.mult,
                    )
                    nc.vector.memset(vh[:, m, :, 64:65], 1.0)

            def emit_heads(bl, post=None):
                b = bt[bl]
                for h in range(8):
                    if post is not None:
                        post(h)
                    po = (h % 2) * 64
                    ch = h // 2
                    qh = b['qhT'][po : po + 64, ch, :]
                    kh = b['khT'][po : po + 64, ch, :]

                    recip = rsp.tile([128, 8], F32, tag="recip", name=f"rc{bl}{h}")
                    pav = [
                        psB.tile([65, 512], F32, tag="av", name=f"pav{bl}{h}_{_n}")
                        for _n in range(2)
                    ]

                    ptgs = []
                    for mg in range(2):
                        ptg = ptp.tile([128, 4, 1024], BF16, tag="pt",
                                       name=f"ptg{bl}{h}_{mg}")
                        ptgs.append(ptg)
                        for ml in range(4):
                            m = mg * 4 + ml
                            ps = psA.tile([128, 1024], F32, tag="s")
                            for n in range(2):
                                nc.tensor.matmul(
                                    ps[:, n * 512 : (n + 1) * 512],
                                    kh[:, m * 128 : (m + 1) * 128],
                                    qh[:, n * 512 : (n + 1) * 512],
                                    start=True,
                                    stop=True,
                                )
                            nc.scalar.activation(
                                ptg[:, ml, :],
                                ps,
                                AF.Exp,
                                bias=biasT[:, bl * 8 + m : bl * 8 + m + 1],
                                scale=0.125,
                            )
                            nc.vector.tensor_mul(
                                ptg[:, ml, :], ptg[:, ml, :], b['maskT'][:, m, :]
                            )
                        for ml in range(4):
                            m = mg * 4 + ml
                            for n in range(2):
                                nc.tensor.matmul(
                                    pav[n],
                                    b['vh'][:, m, h, :],
                                    ptg[:, ml, n * 512 : (n + 1) * 512],
                                    start=(m == 0),
                                    stop=(m == 7),
                                )

                    avs = rsp.tile([65, 1024], F32, tag="avs", name=f"avs{bl}{h}")
                    for n in range(2):
                        nc.vector.tensor_copy(
                            avs[:, n * 512 : (n + 1) * 512], pav[n]
                        )

                    # transpose av result per lq-tile (4 tiles per PSUM group):
                    # [65, 128] -> [128, 65]; col 64 holds softmax row-sums
                    for tg4 in range(2):
                        ptx = psC.tile([128, 4, 65], F32, tag="tr",
                                       name=f"ptx{bl}{h}_{tg4}")
                        for tt in range(4):
                            t = tg4 * 4 + tt
                            nc.tensor.transpose(
                                ptx[:, tt, :],
                                avs[:, t * 128 : (t + 1) * 128],
                                identf[0:65, 0:65],
                            )
                        nc.vector.reciprocal(
                            recip[:, tg4 * 4 : (tg4 + 1) * 4],
                            ptx[:, :, 64],
                        )
                        for tt in range(4):
                            t = tg4 * 4 + tt
                            nc.vector.tensor_scalar(
                                b['xn'][:, t, h * 64 : (h + 1) * 64],
                                ptx[:, tt, 0:64],
                                recip[:, t : t + 1],
                                None,
                                OP.mult,
                            )

                    # transpose P^T back (8 bf16 tiles pack into one PSUM
                    # bank), normalize with the row-sum reciprocal during the
                    # PSUM->SBUF copy, and flush to HBM
                    for t in range(8):
                        pstB = psC.tile([128, 1024], BF16, tag="tr",
                                        padded_shape=[128, 1024],
                                        name=f"pst{bl}{h}_{t}")
                        for i in range(8):
                            nc.tensor.transpose(
                                pstB[:, i * 128 : (i + 1) * 128],
                                ptgs[i // 4][:, i % 4, t * 128 : (t + 1) * 128],
                                identb,
                            )
                        st = stg.tile([128, 1024], BF16, tag="stage",
                                      name=f"st{bl}{h}_{t}")
                        if t % 4 != 3:
                            nc.vector.tensor_scalar(
                                st, pstB, recip[:, t : t + 1], None, OP.mult
                            )
                        else:
                            nc.scalar.mul(st, pstB, recip[:, t : t + 1])
                        nc.gpsimd.dma_start(
                            attn_d[h, bl, t * 128 : (t + 1) * 128, :], st
                        )

            def emit_xretr(bl, j, grp):
                b = bt[bl]
                pst = psA.tile([128, 512], BF16, tag="s",
                               padded_shape=[128, 512], name=f"xr{bl}_{j}_{grp}")
                for tt in range(4):
                    t = grp * 4 + tt
                    nc.tensor.transpose(
                        pst[:, tt * 128 : (tt + 1) * 128],
                        b['xn'][:, t, j * 128 : (j + 1) * 128],
                        identb,
                    )
                nc.scalar.copy(b['outT'][:, j, grp * 512 : (grp + 1) * 512], pst)

            def emit_fc_tile(bl, t):
                b = bt[bl]
                psf = psC.tile([128, 512], F32, tag="tr", name=f"psf{bl}_{t}")
                for j in range(4):
                    nc.tensor.matmul(
                        psf,
                        b['outT'][:, j, t * 128 : (t + 1) * 128],
                        fcw[:, j, :],
                        start=(j == 0),
                        stop=False,
                    )
                nc.tensor.matmul(psf, onesrowb, fcb, start=False, stop=True)

                fco = scr.tile([128, 512], F32, tag="fco", bufs=1, name=f"fco{bl}_{t}")
                nc.scalar.copy(fco, psf)
                sum_t = sp.tile([128, 1], F32, tag="ln", name=f"s{bl}_{t}")
                nc.vector.tensor_reduce(sum_t, psf, axis=AX.X, op=OP.add)
                sumsq = sp.tile([128, 1], F32, tag="ln", name=f"q{bl}_{t}")
                nc.scalar.activation(psf, psf, AF.Square, accum_out=sumsq)
                mean_t = sp.tile([128, 1], F32, tag="ln", name=f"m{bl}_{t}")
                nc.vector.tensor_scalar(mean_t, sum_t, 1.0 / 512.0, None, OP.mult)
                m2 = sp.tile([128, 1], F32, tag="ln", name=f"m2{bl}_{t}")
                nc.vector.tensor_mul(m2, mean_t, mean_t)
                var_t = sp.tile([128, 1], F32, tag="ln", name=f"v{bl}_{t}")
                nc.vector.scalar_tensor_tensor(
                    var_t, sumsq, 1.0 / 512.0, m2, OP.mult, OP.subtract
                )
                t1 = sp.tile([128, 1], F32, tag="ln", name=f"t{bl}_{t}")
                nc.scalar.activation(t1, var_t, AF.Ln, bias=eps, scale=1.0)
                rstd = sp.tile([128, 1], F32, tag="ln", name=f"r{bl}_{t}")
                nc.scalar.activation(rstd, t1, AF.Exp, scale=-0.5)
                y1 = scr.tile([128, 512], F32, tag="y1", name=f"y1{bl}_{t}")
                nc.vector.scalar_tensor_tensor(
                    y1, fco, mean_t, lng_b, OP.subtract, OP.mult
                )
                y2 = scr.tile([128, 512], F32, tag="y1", name=f"y2{bl}_{t}")
                nc.vector.scalar_tensor_tensor(
                    y2, y1, rstd, lnb_b, OP.mult, OP.add
                )
                nc.sync.dma_start(out_d[bl, t * 128 : (t + 1) * 128, :], y2)

            def emit_fc(bl):
                for j in range(4):
                    for grp in range(2):
                        emit_xretr(bl, j, grp)
                for t in range(8):
                    emit_fc_tile(bl, t)

            emit_proj(0)
            emit_heads(0)
            emit_proj(1)

            emit_fc(0)
            emit_heads(1)
            emit_fc(1)

    nc.compile()
    return nc


def kernel(**inputs):
    inp = {k: np.asarray(v) for k, v in inputs.items()}
    q, k, v = inp["q"], inp["k"], inp["v"]
    mask = inp["mask"]
    td = inp["time_diff"].astype(np.float32)

    qT = np.ascontiguousarray(q.astype(np.float32).transpose(0, 2, 1))
    kT = np.ascontiguousarray(k.astype(np.float32).transpose(0, 2, 1))
    import ml_dtypes as _mld
    vT = np.ascontiguousarray(v.transpose(0, 2, 1).astype(_mld.bfloat16))
    import ml_dtypes
    maskT = np.ascontiguousarray(
        (~mask).transpose(0, 2, 1).astype(ml_dtypes.bfloat16)
    )
    tgrid = np.ascontiguousarray(td.reshape(128, 128))

    common = {
        "tp": inp["time_plus"].astype(np.float32).reshape(1, 1),
        "tm": inp["time_mul"].astype(np.float32).reshape(1, 1),
        "wqT": np.ascontiguousarray(inp["Wq"].astype(np.float32).T),
        "wkT": np.ascontiguousarray(inp["Wk"].astype(np.float32).T),
        "wvT": np.ascontiguousarray(inp["Wv"].T.astype(_mld.bfloat16)),
        "fcwT": np.ascontiguousarray(inp["fc_w"].T.astype(ml_dtypes.bfloat16)),
        "fcb": inp["fc_b"].astype(ml_dtypes.bfloat16).reshape(1, D),
        "lng": inp["ln_g"].astype(_mld.bfloat16).reshape(1, D),
        "lnb": inp["ln_b"].astype(_mld.bfloat16).reshape(1, D),
        "ident": np.eye(128, dtype=np.float32),
        "identb": np.eye(128, dtype=ml_dtypes.bfloat16),
        "onesrow": np.ones((1, 128), dtype=np.float32),
        "onesrowb": np.ones((1, 128), dtype=ml_dtypes.bfloat16),
    }

    if "nc" not in _cache:
        _cache["nc"] = _build()
    nc = _cache["nc"]

    in_maps = []
    for c in range(NCORES):
        s = slice(c * BPC, (c + 1) * BPC)
        order = list(range(c * BPC, (c + 1) * BPC)) + [
            b for b in range(B) if not (c * BPC <= b < (c + 1) * BPC)
        ]
        in_maps.append(
            {
                "qT": qT[s],
                "kT": kT[s],
                "vT": vT[s],
                "maskT": maskT[s],
                "tgrid": np.ascontiguousarray(td[order].reshape(128, 128)),
                **common,
            }
        )

    res = bass_utils.run_bass_kernel_spmd(nc, in_maps, core_ids=list(range(NCORES)))

    out = np.empty((B, L, D), np.float32)
    attn = np.empty((H, B, L, L), np.float32)
    for c in range(NCORES):
        r = res.results[c]
        out[c * BPC : (c + 1) * BPC] = r["out_o"]
        attn[:, c * BPC : (c + 1) * BPC] = r["attn_o"]
    return out, attn.reshape(H * B, L, L)
